# revision 4
# baseline (speedup 1.0000x reference)
"""Trainium2 Bass kernel for nn_CrossAttention_block (B=16, Tq=Tk=1024, d_model=24, 4 heads x 6).

Strategy (data-parallel over batch, 2 batches per core on 8 cores):
  - The mask I_m has no effect in the reference (torch masked_fill bug faithfully
    reproduced), so it is never shipped to the device.
  - Everything is computed in "transposed" layout to avoid on-device transposes of
    activations: host supplies X^T / Xen^T per batch; the device computes Q^T/K^T
    with heads replicated at 32-partition offsets so the QK^T score matmuls
    (contraction dim = 6) run 4-way row-tiled on the PE array.
  - Softmax: scores ~ N(0,1) so exp never overflows -> no max subtraction.
    exp runs on the ACT engine with the 1/sqrt(6) scale folded into the
    activation's free affine. Denominators come for free from a ones-column
    appended to V (AV matmuls are 4-way col-tiled, M=7 per head).
  - Normalization: PE-transpose O_aug chunks, reciprocal + per-partition
    tensor_scalar muls on DVE, PE-transpose back, then the Wo projection.
"""

import math
import sys

import numpy as np

if "/opt/trn_rl_repo" not in sys.path:
    sys.path.insert(0, "/opt/trn_rl_repo")

N_CORES = 8
B, T, D = 16, 1024, 24
H, HD = 4, 6
BPC = B // N_CORES  # batches per core
SCALE = 1.0 / math.sqrt(HD)

# matmul dtype for the attention matmuls: "f32", "f32r" or "bf16"
DT_MODE = "f32"

_CACHE = {}


def _build(dt_mode):
    import concourse.tile as tile
    from concourse import bacc, mybir

    f32 = mybir.dt.float32
    if dt_mode == "f32":
        mdt = f32
    elif dt_mode == "f32r":
        mdt = mybir.dt.float32r
    elif dt_mode == "bf16":
        mdt = mybir.dt.bfloat16
    else:
        raise ValueError(dt_mode)
    # dtype used for tiles feeding is_transpose matmuls (must match out tile dtype,
    # and can't mix fp32 with non-fp32 inside one matmul)
    tdt = mybir.dt.float32r if dt_mode == "f32r" else f32

    nc = bacc.Bacc(None)
    XT = nc.declare_dram_parameter("XT", [BPC, D, T], f32, isOutput=False)
    XenT = nc.declare_dram_parameter("XenT", [BPC, D, T], f32, isOutput=False)
    Wq01 = nc.declare_dram_parameter("Wq01", [D, 128], f32, isOutput=False)
    Wq23 = nc.declare_dram_parameter("Wq23", [D, 128], f32, isOutput=False)
    Wk01 = nc.declare_dram_parameter("Wk01", [D, 128], f32, isOutput=False)
    Wk23 = nc.declare_dram_parameter("Wk23", [D, 128], f32, isOutput=False)
    WvT = nc.declare_dram_parameter("WvT", [D, D], f32, isOutput=False)
    WoT = nc.declare_dram_parameter("WoT", [D, D], f32, isOutput=False)
    IDT = nc.declare_dram_parameter("IDT", [128, 128], f32, isOutput=False)
    YT = nc.declare_dram_parameter("YT", [BPC, D, T], f32, isOutput=True)

    with tile.TileContext(nc) as tc:
        from contextlib import ExitStack

        with ExitStack() as es:
            cp = es.enter_context(tc.tile_pool(name="const", bufs=1))
            projp = es.enter_context(tc.tile_pool(name="proj", bufs=4 * BPC))
            vp = es.enter_context(tc.tile_pool(name="vaug", bufs=BPC))
            pp = es.enter_context(tc.tile_pool(name="ptil", bufs=6))
            ep = es.enter_context(tc.tile_pool(name="epi", bufs=3))
            psS = es.enter_context(tc.tile_pool(name="psS", bufs=2, space="PSUM"))
            psO = es.enter_context(tc.tile_pool(name="psO", bufs=2, space="PSUM"))
            psE = es.enter_context(tc.tile_pool(name="psE", bufs=2, space="PSUM"))

            ident = cp.tile([128, 128], tdt, tag="ident")
            nc.sync.dma_start(ident[:], IDT[:])
            wvt = cp.tile([D, D], f32, tag="wvt")
            nc.sync.dma_start(wvt[:], WvT[:])
            wot = cp.tile([D, D], f32, tag="wot")
            nc.sync.dma_start(wot[:], WoT[:])
            wqk = []  # [pair][0]=Wq spread, [1]=Wk spread
            for pair, (dq, dk) in enumerate([(Wq01, Wk01), (Wq23, Wk23)]):
                tq = cp.tile([D, 128], f32, tag=f"wq{pair}")
                nc.sync.dma_start(tq[:], dq[:])
                tk = cp.tile([D, 128], f32, tag=f"wk{pair}")
                nc.sync.dma_start(tk[:], dk[:])
                wqk.append((tq, tk))

            # ---- projections ----
            qts = [[None, None] for _ in range(BPC)]  # [b][pair] -> Q^T tile [128, T]
            kts = [[None, None] for _ in range(BPC)]
            vaugs = []
            for b in range(BPC):
                xt = cp.tile([D, T], f32, tag=f"xt{b}")
                nc.sync.dma_start(xt[:], XT[b])
                xent = cp.tile([D, T], f32, tag=f"xent{b}")
                nc.sync.dma_start(xent[:], XenT[b])

                for pair in range(2):
                    for which, (wsp, dst) in enumerate(
                        [(wqk[pair][0], qts), (wqk[pair][1], kts)]
                    ):
                        sb = projp.tile([128, T], mdt, tag="qkt")
                        src = xt if which == 0 else xent
                        ps = psS.tile([128, 1024], f32, tag="s", name="pjps")
                        for c in range(2):
                            nc.tensor.matmul(
                                ps[:, 512 * c : 512 * (c + 1)],
                                lhsT=wsp[:],
                                rhs=src[:, 512 * c : 512 * (c + 1)],
                                start=True,
                                stop=True,
                            )
                        nc.vector.tensor_copy(sb[:], ps[:])
                        dst[b][pair] = sb

                # V with ones column: [128, 8 chunks x (4 heads x 7)]
                vaug = vp.tile([128, 8 * 28], mdt, tag="vaug")
                nc.vector.memset(vaug[:], 1.0)
                for t in range(8):
                    vps = psS.tile([128, D], f32, tag="s", name="vps")
                    nc.tensor.matmul(
                        vps[:],
                        lhsT=xent[:, 128 * t : 128 * (t + 1)],
                        rhs=wvt[:],
                        start=True,
                        stop=True,
                    )
                    nc.vector.tensor_copy(
                        vaug[:, 28 * t : 28 * (t + 1)].rearrange(
                            "p (h x) -> p h x", h=4
                        )[:, :, 0:6],
                        vps.rearrange("p (h x) -> p h x", h=4),
                    )
                vaugs.append(vaug)

            # ---- attention main loops ----
            for b in range(BPC):
                for qc in range(2):
                    qs = slice(512 * qc, 512 * (qc + 1))
                    o_ps = psO.tile([128, 512], f32, tag="o")
                    for pair in range(2):
                        qt, kt = qts[b][pair], kts[b][pair]
                        for j in range(4):  # ktile pairs
                            stiles = [
                                psS.tile([128, 1024], f32, tag="s", name=f"s{i}")
                                for i in range(2)
                            ]
                            for g in range(4):
                                h_in_pair = g >> 1  # 0 or 1
                                t = 2 * j + (g & 1)
                                nc.tensor.matmul(
                                    stiles[h_in_pair][:, 512 * (g & 1) : 512 * (g & 1) + 512],
                                    lhsT=kt[32 * g : 32 * g + HD, 128 * t : 128 * (t + 1)],
                                    rhs=qt[32 * g : 32 * g + HD, qs],
                                    start=True,
                                    stop=True,
                                    tile_position=(32 * g, 0),
                                )
                            for h_in_pair in range(2):
                                h = 2 * pair + h_in_pair
                                pt = pp.tile([128, 1024], mdt, tag="p")
                                nc.scalar.activation(
                                    pt[:],
                                    stiles[h_in_pair][:],
                                    mybir.ActivationFunctionType.Exp,
                                    scale=SCALE,
                                )
                                for tt in range(2):
                                    t = 2 * j + tt
                                    nc.tensor.matmul(
                                        o_ps[32 * h : 32 * h + 7, :],
                                        lhsT=vaugs[b][:, 28 * t + 7 * h : 28 * t + 7 * h + 7],
                                        rhs=pt[:, 512 * tt : 512 * (tt + 1)],
                                        start=(t == 0),
                                        stop=(t == 7),
                                        tile_position=(0, 32 * h),
                                    )

                    # ---- epilogue: normalize + Wo ----
                    o_sb = ep.tile([128, 512], tdt, tag="osb")
                    nc.vector.tensor_copy(o_sb[:], o_ps[:])
                    on_ps = psE.tile([D, 512], tdt, tag="e", bufs=1)
                    for c in range(4):
                        t_ps = psE.tile([128, 128], tdt, tag="e2", bufs=1)
                        nc.tensor.transpose(
                            t_ps[:], o_sb[:, 128 * c : 128 * (c + 1)], ident[:]
                        )
                        rec = ep.tile([128, 4], f32, tag="rec")
                        nc.vector.reciprocal(rec[:], t_ps[:, 6:128:32])
                        tn = ep.tile([128, D], tdt, tag="tn")
                        for h in range(H):
                            nc.vector.tensor_scalar_mul(
                                tn[:, HD * h : HD * (h + 1)],
                                t_ps[:, 32 * h : 32 * h + HD],
                                rec[:, h : h + 1],
                            )
                        nc.tensor.transpose(
                            on_ps[:, 128 * c : 128 * (c + 1)], tn[:], ident[:]
                        )
                    on_sb = ep.tile([D, 512], f32, tag="onsb")
                    nc.vector.tensor_copy(on_sb[:], on_ps[:])
                    y_ps = psE.tile([D, 512], f32, tag="e", bufs=1)
                    nc.tensor.matmul(
                        y_ps[:], lhsT=wot[:], rhs=on_sb[:], start=True, stop=True
                    )
                    y_sb = ep.tile([D, 512], f32, tag="ysb")
                    nc.vector.tensor_copy(y_sb[:], y_ps[:])
                    nc.sync.dma_start(YT[b][:, qs], y_sb[:])

    nc.compile()
    return nc


def _get_nc():
    if DT_MODE not in _CACHE:
        _CACHE[DT_MODE] = _build(DT_MODE)
    return _CACHE[DT_MODE]


def _spread_w(W, pair):
    out = np.zeros((D, 128), np.float32)
    for g in range(4):
        h = 2 * pair + (1 if g >= 2 else 0)
        out[:, 32 * g : 32 * g + HD] = W[HD * h : HD * (h + 1), :].T
    return out


def kernel(X, X_en, I_m=None, Wq=None, Wk=None, Wv=None, Wo=None):
    from concourse.bass_utils import run_bass_kernel_spmd

    X = np.ascontiguousarray(np.asarray(X, np.float32))
    X_en = np.ascontiguousarray(np.asarray(X_en, np.float32))
    Wq = np.asarray(Wq, np.float32)
    Wk = np.asarray(Wk, np.float32)
    Wv = np.asarray(Wv, np.float32)
    Wo = np.asarray(Wo, np.float32)

    XT_all = np.ascontiguousarray(X.transpose(0, 2, 1))
    XenT_all = np.ascontiguousarray(X_en.transpose(0, 2, 1))
    shared = {
        "Wq01": _spread_w(Wq, 0),
        "Wq23": _spread_w(Wq, 1),
        "Wk01": _spread_w(Wk, 0),
        "Wk23": _spread_w(Wk, 1),
        "WvT": np.ascontiguousarray(Wv.T),
        "WoT": np.ascontiguousarray(Wo.T),
        "IDT": np.eye(128, dtype=np.float32),
    }
    in_maps = [
        {
            "XT": XT_all[BPC * c : BPC * (c + 1)],
            "XenT": XenT_all[BPC * c : BPC * (c + 1)],
            **shared,
        }
        for c in range(N_CORES)
    ]
    nc = _get_nc()
    res = run_bass_kernel_spmd(nc, in_maps, core_ids=list(range(N_CORES)))
    Y = np.concatenate(
        [r["YT"].transpose(0, 2, 1) for r in res.results], axis=0
    )
    return np.ascontiguousarray(Y, dtype=np.float32)


# revision 5
# speedup vs baseline: 11.5177x; 11.5177x over previous
"""Trainium2 Bass kernel for nn_CrossAttention_block (B=16, Tq=Tk=1024, d_model=24, 4 heads x 6).

Strategy (data-parallel over batch, 2 batches per core on 8 cores):
  - The mask I_m has no effect in the reference (torch masked_fill bug faithfully
    reproduced), so it is never shipped to the device.
  - Everything is computed in "transposed" layout to avoid on-device transposes of
    activations: host supplies X^T / Xen^T per batch; the device computes Q^T/K^T
    with heads replicated at 32-partition offsets so the QK^T score matmuls
    (contraction dim = 6) run 4-way row-tiled on the PE array.
  - Softmax: scores ~ N(0,1) so exp never overflows -> no max subtraction.
    exp runs on the ACT engine with the 1/sqrt(6) scale folded into the
    activation's free affine. Denominators come for free from a ones-column
    appended to V (AV matmuls are 4-way col-tiled, M=7 per head).
  - Normalization: PE-transpose O_aug chunks, reciprocal + per-partition
    tensor_scalar muls on DVE, PE-transpose back, then the Wo projection.
"""

import math
import sys

import numpy as np

if "/opt/trn_rl_repo" not in sys.path:
    sys.path.insert(0, "/opt/trn_rl_repo")

N_CORES = 8
B, T, D = 16, 1024, 24
H, HD = 4, 6
BPC = B // N_CORES  # batches per core
SCALE = 1.0 / math.sqrt(HD)

# matmul dtype for the attention matmuls: "f32", "f32r" or "bf16"
DT_MODE = "f32"
# number of times the attention body is emitted (timing experiments only)
REPEAT = 1

_CACHE = {}


def _build(dt_mode, repeat=1):
    import concourse.tile as tile
    from concourse import bacc, mybir

    f32 = mybir.dt.float32
    if dt_mode == "f32":
        mdt = f32
    elif dt_mode == "f32r":
        mdt = mybir.dt.float32r
    elif dt_mode == "bf16":
        mdt = mybir.dt.bfloat16
    else:
        raise ValueError(dt_mode)
    # dtype used for tiles feeding is_transpose matmuls (must match out tile dtype,
    # and can't mix fp32 with non-fp32 inside one matmul)
    tdt = mybir.dt.float32r if dt_mode == "f32r" else f32

    nc = bacc.Bacc(None)
    XT = nc.declare_dram_parameter("XT", [BPC, D, T], f32, isOutput=False)
    XenT = nc.declare_dram_parameter("XenT", [BPC, D, T], f32, isOutput=False)
    Wq01 = nc.declare_dram_parameter("Wq01", [D, 128], f32, isOutput=False)
    Wq23 = nc.declare_dram_parameter("Wq23", [D, 128], f32, isOutput=False)
    Wk01 = nc.declare_dram_parameter("Wk01", [D, 128], f32, isOutput=False)
    Wk23 = nc.declare_dram_parameter("Wk23", [D, 128], f32, isOutput=False)
    WvT = nc.declare_dram_parameter("WvT", [D, D], f32, isOutput=False)
    WoT = nc.declare_dram_parameter("WoT", [D, D], f32, isOutput=False)
    IDT = nc.declare_dram_parameter("IDT", [128, 128], f32, isOutput=False)
    YT = nc.declare_dram_parameter("YT", [BPC, D, T], f32, isOutput=True)

    with tile.TileContext(nc) as tc:
        from contextlib import ExitStack

        with ExitStack() as es:
            cp = es.enter_context(tc.tile_pool(name="const", bufs=1))
            projp = es.enter_context(tc.tile_pool(name="proj", bufs=4 * BPC))
            vp = es.enter_context(tc.tile_pool(name="vaug", bufs=BPC))
            pp = es.enter_context(tc.tile_pool(name="ptil", bufs=6))
            ep = es.enter_context(tc.tile_pool(name="epi", bufs=3))
            psS = es.enter_context(tc.tile_pool(name="psS", bufs=2, space="PSUM"))
            psO = es.enter_context(tc.tile_pool(name="psO", bufs=2, space="PSUM"))
            psE = es.enter_context(tc.tile_pool(name="psE", bufs=2, space="PSUM"))

            ident = cp.tile([128, 128], tdt, tag="ident")
            nc.sync.dma_start(ident[:], IDT[:])
            wvt = cp.tile([D, D], f32, tag="wvt")
            nc.sync.dma_start(wvt[:], WvT[:])
            wot = cp.tile([D, D], f32, tag="wot")
            nc.sync.dma_start(wot[:], WoT[:])
            wqk = []  # [pair][0]=Wq spread, [1]=Wk spread
            for pair, (dq, dk) in enumerate([(Wq01, Wk01), (Wq23, Wk23)]):
                tq = cp.tile([D, 128], f32, tag=f"wq{pair}")
                nc.sync.dma_start(tq[:], dq[:])
                tk = cp.tile([D, 128], f32, tag=f"wk{pair}")
                nc.sync.dma_start(tk[:], dk[:])
                wqk.append((tq, tk))

            # ---- projections ----
            qts = [[None, None] for _ in range(BPC)]  # [b][pair] -> Q^T tile [128, T]
            kts = [[None, None] for _ in range(BPC)]
            vaugs = []
            for b in range(BPC):
                xt = cp.tile([D, T], f32, tag=f"xt{b}")
                nc.sync.dma_start(xt[:], XT[b])
                xent = cp.tile([D, T], f32, tag=f"xent{b}")
                nc.sync.dma_start(xent[:], XenT[b])

                for pair in range(2):
                    for which, (wsp, dst) in enumerate(
                        [(wqk[pair][0], qts), (wqk[pair][1], kts)]
                    ):
                        sb = projp.tile([128, T], mdt, tag="qkt")
                        src = xt if which == 0 else xent
                        ps = psS.tile([128, 1024], f32, tag="s", name="pjps")
                        for c in range(2):
                            nc.tensor.matmul(
                                ps[:, 512 * c : 512 * (c + 1)],
                                lhsT=wsp[:],
                                rhs=src[:, 512 * c : 512 * (c + 1)],
                                start=True,
                                stop=True,
                            )
                        nc.vector.tensor_copy(sb[:], ps[:])
                        dst[b][pair] = sb

                # V with ones column: [128, 8 chunks x (4 heads x 7)]
                vaug = vp.tile([128, 8 * 28], mdt, tag="vaug")
                nc.vector.memset(vaug[:], 1.0)
                for t in range(8):
                    vps = psS.tile([128, D], f32, tag="s", name="vps")
                    nc.tensor.matmul(
                        vps[:],
                        lhsT=xent[:, 128 * t : 128 * (t + 1)],
                        rhs=wvt[:],
                        start=True,
                        stop=True,
                    )
                    nc.vector.tensor_copy(
                        vaug[:, 28 * t : 28 * (t + 1)].rearrange(
                            "p (h x) -> p h x", h=4
                        )[:, :, 0:6],
                        vps.rearrange("p (h x) -> p h x", h=4),
                    )
                vaugs.append(vaug)

            # ---- attention main loops ----
            for _rep in range(repeat):
              for b in range(BPC):
                for qc in range(2):
                    qs = slice(512 * qc, 512 * (qc + 1))
                    o_ps = psO.tile([128, 512], f32, tag="o")
                    for pair in range(2):
                        qt, kt = qts[b][pair], kts[b][pair]
                        for j in range(4):  # ktile pairs
                            stiles = [
                                psS.tile([128, 1024], f32, tag="s", name=f"s{i}")
                                for i in range(2)
                            ]
                            for g in range(4):
                                h_in_pair = g >> 1  # 0 or 1
                                t = 2 * j + (g & 1)
                                nc.tensor.matmul(
                                    stiles[h_in_pair][:, 512 * (g & 1) : 512 * (g & 1) + 512],
                                    lhsT=kt[32 * g : 32 * g + HD, 128 * t : 128 * (t + 1)],
                                    rhs=qt[32 * g : 32 * g + HD, qs],
                                    start=True,
                                    stop=True,
                                    tile_position=(32 * g, 0),
                                )
                            for h_in_pair in range(2):
                                h = 2 * pair + h_in_pair
                                pt = pp.tile([128, 1024], mdt, tag="p")
                                nc.scalar.activation(
                                    pt[:],
                                    stiles[h_in_pair][:],
                                    mybir.ActivationFunctionType.Exp,
                                    scale=SCALE,
                                )
                                for tt in range(2):
                                    t = 2 * j + tt
                                    nc.tensor.matmul(
                                        o_ps[32 * h : 32 * h + 7, :],
                                        lhsT=vaugs[b][:, 28 * t + 7 * h : 28 * t + 7 * h + 7],
                                        rhs=pt[:, 512 * tt : 512 * (tt + 1)],
                                        start=(t == 0),
                                        stop=(t == 7),
                                        tile_position=(0, 32 * h),
                                    )

                    # ---- epilogue: normalize + Wo ----
                    o_sb = ep.tile([128, 512], tdt, tag="osb")
                    nc.vector.tensor_copy(o_sb[:], o_ps[:])
                    on_ps = psE.tile([D, 512], tdt, tag="e", bufs=1)
                    for c in range(4):
                        t_ps = psE.tile([128, 128], tdt, tag="e2", bufs=1)
                        nc.tensor.transpose(
                            t_ps[:], o_sb[:, 128 * c : 128 * (c + 1)], ident[:]
                        )
                        rec = ep.tile([128, 4], f32, tag="rec")
                        nc.vector.reciprocal(rec[:], t_ps[:, 6:128:32])
                        tn = ep.tile([128, D], tdt, tag="tn")
                        for h in range(H):
                            nc.vector.tensor_scalar_mul(
                                tn[:, HD * h : HD * (h + 1)],
                                t_ps[:, 32 * h : 32 * h + HD],
                                rec[:, h : h + 1],
                            )
                        nc.tensor.transpose(
                            on_ps[:, 128 * c : 128 * (c + 1)], tn[:], ident[:]
                        )
                    on_sb = ep.tile([D, 512], f32, tag="onsb")
                    nc.vector.tensor_copy(on_sb[:], on_ps[:])
                    y_ps = psE.tile([D, 512], f32, tag="e", bufs=1)
                    nc.tensor.matmul(
                        y_ps[:], lhsT=wot[:], rhs=on_sb[:], start=True, stop=True
                    )
                    y_sb = ep.tile([D, 512], f32, tag="ysb")
                    nc.vector.tensor_copy(y_sb[:], y_ps[:])
                    nc.sync.dma_start(YT[b][:, qs], y_sb[:])

    nc.compile()
    return nc


def _get_nc():
    key = (DT_MODE, REPEAT)
    if key not in _CACHE:
        _CACHE[key] = _build(DT_MODE, REPEAT)
    return _CACHE[key]


def _spread_w(W, pair):
    out = np.zeros((D, 128), np.float32)
    for g in range(4):
        h = 2 * pair + (1 if g >= 2 else 0)
        out[:, 32 * g : 32 * g + HD] = W[HD * h : HD * (h + 1), :].T
    return out


def kernel(X, X_en, I_m=None, Wq=None, Wk=None, Wv=None, Wo=None):
    from concourse.bass_utils import run_bass_kernel_spmd

    X = np.ascontiguousarray(np.asarray(X, np.float32))
    X_en = np.ascontiguousarray(np.asarray(X_en, np.float32))
    Wq = np.asarray(Wq, np.float32)
    Wk = np.asarray(Wk, np.float32)
    Wv = np.asarray(Wv, np.float32)
    Wo = np.asarray(Wo, np.float32)

    XT_all = np.ascontiguousarray(X.transpose(0, 2, 1))
    XenT_all = np.ascontiguousarray(X_en.transpose(0, 2, 1))
    shared = {
        "Wq01": _spread_w(Wq, 0),
        "Wq23": _spread_w(Wq, 1),
        "Wk01": _spread_w(Wk, 0),
        "Wk23": _spread_w(Wk, 1),
        "WvT": np.ascontiguousarray(Wv.T),
        "WoT": np.ascontiguousarray(Wo.T),
        "IDT": np.eye(128, dtype=np.float32),
    }
    in_maps = [
        {
            "XT": XT_all[BPC * c : BPC * (c + 1)],
            "XenT": XenT_all[BPC * c : BPC * (c + 1)],
            **shared,
        }
        for c in range(N_CORES)
    ]
    nc = _get_nc()
    res = run_bass_kernel_spmd(nc, in_maps, core_ids=list(range(N_CORES)))
    Y = np.concatenate(
        [r["YT"].transpose(0, 2, 1) for r in res.results], axis=0
    )
    return np.ascontiguousarray(Y, dtype=np.float32)


# revision 10
# speedup vs baseline: 404.2400x; 35.0971x over previous
"""Trainium2 Bass kernel for nn_CrossAttention_block (B=16, Tq=Tk=1024, d_model=24, 4 heads x 6).

Strategy (data-parallel over batch, 2 batches per core on 8 cores):
  - The mask I_m has no effect in the reference (torch masked_fill bug faithfully
    reproduced), so it is never shipped to the device.
  - Everything is computed in "transposed" layout to avoid on-device transposes of
    activations: host supplies X^T / Xen^T per batch; the device computes Q^T/K^T
    with heads replicated at 32-partition offsets so the QK^T score matmuls
    (contraction dim = 6) run 4-way row-tiled on the PE array.
  - Softmax: scores ~ N(0,1) so exp never overflows -> no max subtraction.
    exp runs on the ACT engine with the 1/sqrt(6) scale folded into the
    activation's free affine. Denominators come for free from a ones-column
    appended to V (AV matmuls are 4-way col-tiled, M=7 per head).
  - Normalization: PE-transpose O_aug chunks, reciprocal + per-partition
    tensor_scalar muls on DVE, PE-transpose back, then the Wo projection.
"""

import math
import sys

import numpy as np

if "/opt/trn_rl_repo" not in sys.path:
    sys.path.insert(0, "/opt/trn_rl_repo")

N_CORES = 8
B, T, D = 16, 1024, 24
H, HD = 4, 6
BPC = B // N_CORES  # batches per core
SCALE = 1.0 / math.sqrt(HD)

# matmul dtype for the attention matmuls: "f32", "f32r" or "bf16"
DT_MODE = "f32"
# number of times the attention body is emitted (timing experiments only)
REPEAT = 1

_CACHE = {}


def _build(dt_mode, repeat=1):
    import concourse.tile as tile
    from concourse import bacc, mybir

    f32 = mybir.dt.float32
    bf16 = mybir.dt.bfloat16
    if dt_mode in ("f32", "f32r"):
        mdt_qk, mdt_av = f32, f32
    elif dt_mode == "bf16":
        mdt_qk, mdt_av = bf16, bf16
    elif dt_mode == "mixed":
        # scores in f32 (exp amplifies score error); AV in bf16 (rounding
        # averages out over the 1024-term softmax sums)
        mdt_qk, mdt_av = f32, bf16
    else:
        raise ValueError(dt_mode)
    tdt = f32

    def dv(ap):
        return ap

    def mm(ap):
        # f32r view for the big attention matmuls: single-pass PE fp32 (4x faster)
        return ap.bitcast(mybir.dt.float32r) if dt_mode == "f32r" else ap

    nc = bacc.Bacc(None)
    XT = nc.declare_dram_parameter("XT", [BPC, D, T], f32, isOutput=False)
    XenT = nc.declare_dram_parameter("XenT", [BPC, D, T], f32, isOutput=False)
    Wq01 = nc.declare_dram_parameter("Wq01", [D, 128], f32, isOutput=False)
    Wq23 = nc.declare_dram_parameter("Wq23", [D, 128], f32, isOutput=False)
    Wk01 = nc.declare_dram_parameter("Wk01", [D, 128], f32, isOutput=False)
    Wk23 = nc.declare_dram_parameter("Wk23", [D, 128], f32, isOutput=False)
    WvT = nc.declare_dram_parameter("WvT", [D, D], f32, isOutput=False)
    WoT = nc.declare_dram_parameter("WoT", [D, D], f32, isOutput=False)
    IDT = nc.declare_dram_parameter("IDT", [128, 128], f32, isOutput=False)
    YT = nc.declare_dram_parameter("YT", [BPC, D, T], f32, isOutput=True)

    with tile.TileContext(nc) as tc:
        from contextlib import ExitStack

        with ExitStack() as es:
            cp = es.enter_context(tc.tile_pool(name="const", bufs=1))
            projp = es.enter_context(tc.tile_pool(name="proj", bufs=4 * BPC))
            vp = es.enter_context(tc.tile_pool(name="vaug", bufs=BPC))
            pp = es.enter_context(tc.tile_pool(name="ptil", bufs=6))
            ep = es.enter_context(tc.tile_pool(name="epi", bufs=3))
            psS = es.enter_context(tc.tile_pool(name="psS", bufs=2, space="PSUM"))
            psO = es.enter_context(tc.tile_pool(name="psO", bufs=2, space="PSUM"))
            psE = es.enter_context(tc.tile_pool(name="psE", bufs=2, space="PSUM"))

            ident = cp.tile([128, 128], tdt, tag="ident")
            nc.sync.dma_start(ident[:], IDT[:])
            wvt = cp.tile([D, D], f32, tag="wvt")
            nc.sync.dma_start(wvt[:], WvT[:])
            wot = cp.tile([D, D], f32, tag="wot")
            nc.sync.dma_start(wot[:], WoT[:])
            wqk = []  # [pair][0]=Wq spread, [1]=Wk spread
            for pair, (dq, dk) in enumerate([(Wq01, Wk01), (Wq23, Wk23)]):
                tq = cp.tile([D, 128], f32, tag=f"wq{pair}")
                nc.sync.dma_start(tq[:], dq[:])
                tk = cp.tile([D, 128], f32, tag=f"wk{pair}")
                nc.sync.dma_start(tk[:], dk[:])
                wqk.append((tq, tk))

            # ---- projections ----
            qts = [[None, None] for _ in range(BPC)]  # [b][pair] -> Q^T tile [128, T]
            kts = [[None, None] for _ in range(BPC)]
            vaugs = []
            for b in range(BPC):
                xt = cp.tile([D, T], f32, tag=f"xt{b}")
                nc.sync.dma_start(xt[:], XT[b])
                xent = cp.tile([D, T], f32, tag=f"xent{b}")
                nc.sync.dma_start(xent[:], XenT[b])

                for pair in range(2):
                    for which, (wsp, dst) in enumerate(
                        [(wqk[pair][0], qts), (wqk[pair][1], kts)]
                    ):
                        sb = projp.tile([128, T], mdt_qk, tag="qkt")
                        src = xt if which == 0 else xent
                        ps = psS.tile([128, 1024], f32, tag="s", name="pjps")
                        for c in range(2):
                            nc.tensor.matmul(
                                ps[:, 512 * c : 512 * (c + 1)],
                                lhsT=wsp[:],
                                rhs=src[:, 512 * c : 512 * (c + 1)],
                                start=True,
                                stop=True,
                            )
                        nc.vector.tensor_copy(dv(sb[:]), ps[:])
                        dst[b][pair] = sb

                # V with ones column: [128, 8 chunks x (4 heads x 7)]
                vaug = vp.tile([128, 8 * 28], mdt_av, tag="vaug")
                nc.vector.memset(dv(vaug[:]), 1.0)
                for t in range(8):
                    vps = psS.tile([128, D], f32, tag="s", name="vps")
                    nc.tensor.matmul(
                        vps[:],
                        lhsT=xent[:, 128 * t : 128 * (t + 1)],
                        rhs=wvt[:],
                        start=True,
                        stop=True,
                    )
                    nc.vector.tensor_copy(
                        dv(vaug[:, 28 * t : 28 * (t + 1)]).rearrange(
                            "p (h x) -> p h x", h=4
                        )[:, :, 0:6],
                        vps.rearrange("p (h x) -> p h x", h=4),
                    )
                vaugs.append(vaug)

            # ---- attention main loops ----
            for _rep in range(repeat):
              for b in range(BPC):
                for qc in range(2):
                    qs = slice(512 * qc, 512 * (qc + 1))
                    o_ps = psO.tile([128, 512], f32, tag="o")
                    for pair in range(2):
                        qt, kt = qts[b][pair], kts[b][pair]
                        for j in range(4):  # ktile pairs
                            stiles = [
                                psS.tile([128, 1024], f32, tag="s", name=f"s{i}")
                                for i in range(2)
                            ]
                            for g in range(4):
                                h_in_pair = g >> 1  # 0 or 1
                                t = 2 * j + (g & 1)
                                nc.tensor.matmul(
                                    stiles[h_in_pair][:, 512 * (g & 1) : 512 * (g & 1) + 512],
                                    lhsT=mm(kt[32 * g : 32 * g + HD, 128 * t : 128 * (t + 1)]),
                                    rhs=mm(qt[32 * g : 32 * g + HD, qs]),
                                    start=True,
                                    stop=True,
                                    tile_position=(32 * g, 0),
                                )
                            for h_in_pair in range(2):
                                h = 2 * pair + h_in_pair
                                pt = pp.tile([128, 1024], mdt_av, tag="p")
                                nc.scalar.activation(
                                    dv(pt[:]),
                                    stiles[h_in_pair][:],
                                    mybir.ActivationFunctionType.Exp,
                                    scale=SCALE,
                                )
                                for tt in range(2):
                                    t = 2 * j + tt
                                    nc.tensor.matmul(
                                        o_ps[32 * h : 32 * h + 7, :],
                                        lhsT=mm(vaugs[b][:, 28 * t + 7 * h : 28 * t + 7 * h + 7]),
                                        rhs=mm(pt[:, 512 * tt : 512 * (tt + 1)]),
                                        start=(t == 0),
                                        stop=(t == 7),
                                        tile_position=(0, 32 * h),
                                    )

                    # ---- epilogue: normalize + Wo ----
                    o_sb = ep.tile([128, 512], tdt, tag="osb")
                    nc.vector.tensor_copy(dv(o_sb[:]), o_ps[:])
                    on_ps = psE.tile([D, 512], tdt, tag="e", bufs=1)
                    for c in range(4):
                        t_ps = psE.tile([128, 128], tdt, tag="e2", bufs=1)
                        nc.tensor.transpose(
                            t_ps[:], o_sb[:, 128 * c : 128 * (c + 1)], ident[:]
                        )
                        rec = ep.tile([128, 4], f32, tag="rec")
                        nc.vector.reciprocal(rec[:], dv(t_ps[:, 6:128:32]))
                        tn = ep.tile([128, D], tdt, tag="tn")
                        for h in range(H):
                            nc.vector.tensor_scalar_mul(
                                dv(tn[:, HD * h : HD * (h + 1)]),
                                dv(t_ps[:, 32 * h : 32 * h + HD]),
                                rec[:, h : h + 1],
                            )
                        nc.tensor.transpose(
                            on_ps[:, 128 * c : 128 * (c + 1)], tn[:], ident[:]
                        )
                    on_sb = ep.tile([D, 512], f32, tag="onsb")
                    nc.vector.tensor_copy(on_sb[:], dv(on_ps[:]))
                    y_ps = psE.tile([D, 512], f32, tag="e", bufs=1)
                    nc.tensor.matmul(
                        y_ps[:], lhsT=wot[:], rhs=on_sb[:], start=True, stop=True
                    )
                    y_sb = ep.tile([D, 512], f32, tag="ysb")
                    nc.vector.tensor_copy(y_sb[:], y_ps[:])
                    nc.sync.dma_start(YT[b][:, qs], y_sb[:])

    nc.compile()
    return nc


def _make_runner(nc, n_cores=N_CORES):
    """Build the sharded PJRT callable once; reuse across kernel() calls so
    repeat calls skip retracing and NEFF reload."""
    import jax
    from jax.experimental.shard_map import shard_map
    from jax.sharding import Mesh, NamedSharding, PartitionSpec

    from concourse import bass2jax, mybir

    bass2jax.install_neuronx_cc_hook()
    partition_name = nc.partition_id_tensor.name if nc.partition_id_tensor else None

    in_names, out_names, out_avals, zero_outs = [], [], [], []
    for alloc in nc.m.functions[0].allocations:
        if not isinstance(alloc, mybir.MemoryLocationSet):
            continue
        name = alloc.memorylocations[0].name
        if alloc.kind == "ExternalInput":
            if name != partition_name:
                in_names.append(name)
        elif alloc.kind == "ExternalOutput":
            out_names.append(name)
            shape = tuple(alloc.tensor_shape)
            dtype = mybir.dt.np(alloc.dtype)
            out_avals.append(jax.core.ShapedArray(shape, dtype))
            zero_outs.append(np.zeros(shape, dtype))
    n_params = len(in_names)
    n_outs = len(out_avals)
    all_in_names = list(in_names) + list(out_names)
    if partition_name is not None:
        all_in_names.append(partition_name)

    def _body(*args):
        operands = list(args)
        if partition_name is not None:
            operands.append(bass2jax.partition_id_tensor())
        return tuple(
            bass2jax._bass_exec_p.bind(
                *operands,
                out_avals=tuple(out_avals),
                in_names=tuple(all_in_names),
                out_names=tuple(out_names),
                lowering_input_output_aliases=(),
                sim_require_finite=True,
                sim_require_nnan=True,
                nc=nc,
            )
        )

    devices = jax.devices()[:n_cores]
    mesh = Mesh(np.asarray(devices), ("core",))
    in_specs = (PartitionSpec("core"),) * (n_params + n_outs)
    out_specs = (PartitionSpec("core"),) * len(out_names)
    fn = jax.jit(
        shard_map(_body, mesh=mesh, in_specs=in_specs, out_specs=out_specs,
                  check_rep=False),
        keep_unused=True,
    )
    sharding = NamedSharding(mesh, PartitionSpec("core"))
    concat_zeros = [
        jax.device_put(
            np.zeros((n_cores * z.shape[0], *z.shape[1:]), z.dtype), sharding
        )
        for z in zero_outs
    ]

    def run(in_maps):
        staged = [
            jax.device_put(
                np.concatenate(
                    [np.asarray(in_maps[c][nm]) for c in range(n_cores)], axis=0
                ),
                sharding,
            )
            for nm in in_names
        ]
        out_arrs = [np.asarray(a) for a in fn(*staged, *concat_zeros)]
        return [
            {
                name: out_arrs[i].reshape(n_cores, *out_avals[i].shape)[c]
                for i, name in enumerate(out_names)
            }
            for c in range(n_cores)
        ]

    return run


def _get_runner():
    key = (DT_MODE, REPEAT)
    if key not in _CACHE:
        _CACHE[key] = _make_runner(_build(DT_MODE, REPEAT))
    return _CACHE[key]


def _spread_w(W, pair):
    out = np.zeros((D, 128), np.float32)
    for g in range(4):
        h = 2 * pair + (1 if g >= 2 else 0)
        out[:, 32 * g : 32 * g + HD] = W[HD * h : HD * (h + 1), :].T
    return out


def kernel(X, X_en, I_m=None, Wq=None, Wk=None, Wv=None, Wo=None):
    X = np.ascontiguousarray(np.asarray(X, np.float32))
    X_en = np.ascontiguousarray(np.asarray(X_en, np.float32))
    Wq = np.asarray(Wq, np.float32)
    Wk = np.asarray(Wk, np.float32)
    Wv = np.asarray(Wv, np.float32)
    Wo = np.asarray(Wo, np.float32)

    XT_all = np.ascontiguousarray(X.transpose(0, 2, 1))
    XenT_all = np.ascontiguousarray(X_en.transpose(0, 2, 1))
    shared = {
        "Wq01": _spread_w(Wq, 0),
        "Wq23": _spread_w(Wq, 1),
        "Wk01": _spread_w(Wk, 0),
        "Wk23": _spread_w(Wk, 1),
        "WvT": np.ascontiguousarray(Wv.T),
        "WoT": np.ascontiguousarray(Wo.T),
        "IDT": np.eye(128, dtype=np.float32),
    }
    in_maps = [
        {
            "XT": XT_all[BPC * c : BPC * (c + 1)],
            "XenT": XenT_all[BPC * c : BPC * (c + 1)],
            **shared,
        }
        for c in range(N_CORES)
    ]
    res = _get_runner()(in_maps)
    Y = np.concatenate([r["YT"].transpose(0, 2, 1) for r in res], axis=0)
    return np.ascontiguousarray(Y, dtype=np.float32)
